# revision 1
# baseline (speedup 1.0000x reference)
"""GCN (2-layer, PyG GCNConv-style) on 8 Trainium2 NeuronCores.

Strategy (per spec sharding hint):
  - Destination nodes sharded 8 ways (6272 = 49*128 dst slots / core, last
    core partially filled); edges partitioned by destination shard on host.
  - Small weights replicated; every core computes the full first-layer
    linear transform h' = dinv * (x @ W1) (cheap) so the per-edge gather
    is purely local — no halo traffic for layer 1.
  - Per-edge rows fetched with dma_gather (int16 indices, 256B elements);
    the node->row permutation rowid = (n%128)*392 + n//128 makes rows for
    both layers live in one shared index space, so layer 1 and layer 2 use
    identical index tables and selection data. int16 range forces a
    low(<32768)/high row split: each dst tile accumulates its low chunks
    and high chunks in two PSUM passes joined in the epilogue.
  - Aggregation: edges grouped into 128-edge chunks per 128-dst-node tile;
    gathered rows are segment-summed on the PE via one-hot selection-matrix
    matmuls (sel[e,d] = (dst_e == d)) accumulating in PSUM.
  - Between layers, the [6272,16] shard features are AllGathered (1.6 MB),
    then expanded locally into the padded gather layout.
  - log_softmax epilogue on-chip; output sharded by dst, host concatenates.
"""

import os
import sys

import numpy as np

for _p in ("/opt/trn_rl_repo", "/root/.axon_site/_ro/trn_rl_repo"):
    if os.path.isdir(_p) and _p not in sys.path:
        sys.path.insert(0, _p)

import ml_dtypes  # noqa: E402
import concourse.bass as bass  # noqa: E402
import concourse.mybir as mybir  # noqa: E402
import concourse.tile as tile  # noqa: E402
from concourse.bass_utils import run_bass_kernel_spmd  # noqa: E402
from concourse.masks import make_identity  # noqa: E402
from concourse import library_config  # noqa: E402
import concourse.bass_isa as bass_isa  # noqa: E402

# ---------------- static problem config (hardcoded per contract) -------------
N = 50000
E = 800000
F = 64          # F_IN == F_HID
FO = 16         # F_OUT
NCORES = 8
P = 128
NT = 49                   # dst tiles per core
SH = NT * P               # 6272 dst slots per core (last core: 6096 real)
NTH = 392                 # node tiles for h' (50176 = 392*128 padded nodes)
NPAD = NTH * P            # 50176
XH = NTH // 2             # 196 tiles per half
XCOLS = XH * P            # 25088
G = 64                    # chunks per dma_gather instruction (8192 edges)
EPR = 128                 # padded row length (elements, bf16) = 256 bytes
LOWROWS = 32768           # int16 index range split

BF16 = ml_dtypes.bfloat16

_CACHE = {}


def _rowid(n):
    return (n % P) * NTH + n // P


def _pack_idx16(vals):
    """Edge-position-ordered values [Ctot*128] -> int16 idx table [128, Ctot*8].

    dma_gather reads index k from (partition k%16, col k//16), replicated
    across the 8 q7 cores (partition stripes of 16).
    """
    k = np.arange(vals.size)
    tbl = np.zeros((16, vals.size // 16), dtype=np.int16)
    tbl[k % 16, k // 16] = vals.astype(np.int16)
    return np.tile(tbl, (8, 1))


def _preprocess(x, edge_index, W1, b1, W2, b2):
    """Host-side graph preprocessing + input staging for all 8 cores."""
    src = np.concatenate([np.asarray(edge_index[0]), np.arange(N, dtype=np.int64)])
    dst = np.concatenate([np.asarray(edge_index[1]), np.arange(N, dtype=np.int64)])

    deg = np.bincount(dst, minlength=N).astype(np.float64)
    dinv = np.where(deg > 0, 1.0 / np.sqrt(deg), 0.0).astype(np.float32)

    core = dst // SH
    rows = _rowid(src)
    lo = rows < LOWROWS

    per_core = []
    cnt_lo = np.zeros((NCORES, NT), dtype=np.int64)
    cnt_hi = np.zeros((NCORES, NT), dtype=np.int64)
    for c in range(NCORES):
        m = core == c
        r_c = rows[m]
        d_c = dst[m] - c * SH
        t_c = d_c // P
        lo_c = lo[m]
        for t in range(NT):
            cnt_lo[c, t] = np.count_nonzero((t_c == t) & lo_c)
            cnt_hi[c, t] = np.count_nonzero((t_c == t) & ~lo_c)
        per_core.append((r_c, d_c, t_c, lo_c))

    # uniform chunk counts across cores (SPMD: one program for all cores)
    C_lo = ((cnt_lo.max(axis=0) + P - 1) // P).astype(np.int64)
    C_hi = ((cnt_hi.max(axis=0) + P - 1) // P).astype(np.int64)
    off_lo = np.concatenate([[0], np.cumsum(C_lo)])
    Clo_tot = int(off_lo[-1])
    off_hi = Clo_tot + np.concatenate([[0], np.cumsum(C_hi)])
    Ctot = int(off_hi[-1])

    # edge-position-ordered value arrays; position k = chunk*128 + lane
    idxv = np.zeros((NCORES, Ctot * P), dtype=np.int64)
    selv = np.full((NCORES, P, Ctot), -1.0, dtype=np.float32)
    for c in range(NCORES):
        r_c, d_c, t_c, lo_c = per_core[c]
        for t in range(NT):
            for is_lo in (True, False):
                m = (t_c == t) & (lo_c == is_lo)
                rr = r_c[m]
                dd = (d_c[m] % P).astype(np.float32)
                base = int(off_lo[t]) if is_lo else int(off_hi[t])
                i = np.arange(rr.size)
                pos = (base + i // P) * P + (i % P)
                idxv[c, pos] = rr - (0 if is_lo else LOWROWS)
                selv[c, i % P, base + i // P] = dd
    idx16 = np.stack([_pack_idx16(idxv[c]) for c in range(NCORES)])

    # xpack [128, XCOLS] bf16: partition (a*64+f), col j  <->  x_pad[a*XCOLS+j, f]
    xpad = np.zeros((NPAD, F), dtype=np.float32)
    xpad[:N] = np.asarray(x, dtype=np.float32)
    xpack = (
        xpad.reshape(2, XCOLS, F).transpose(0, 2, 1).reshape(P, XCOLS).astype(BF16)
    )

    dinv_h = np.zeros((P, NTH), dtype=np.float32)
    nn = np.arange(N)
    dinv_h[nn % P, nn // P] = dinv
    dinv_s = np.zeros((NCORES, P, NT), dtype=np.float32)
    for c in range(NCORES):
        ll = np.arange(min(SH, N - c * SH))
        dinv_s[c, ll % P, ll // P] = dinv[c * SH : c * SH + ll.size]

    common = {
        "xpack": xpack,
        "w1": np.concatenate([np.asarray(W1, np.float32)] * 2, axis=0).astype(BF16),
        "w2": np.asarray(W2, dtype=np.float32).astype(BF16),
        "b1r": np.broadcast_to(np.asarray(b1, np.float32), (P, F)).copy(),
        "b2r": np.broadcast_to(np.asarray(b2, np.float32), (P, FO)).copy(),
        "dinvh": dinv_h,
    }
    in_maps = []
    for c in range(NCORES):
        m = dict(common)
        m["dinvs"] = dinv_s[c]
        m["idx16"] = idx16[c]
        m["selv"] = selv[c]
        in_maps.append(m)

    cot_lo = [list(range(int(off_lo[t]), int(off_lo[t + 1]))) for t in range(NT)]
    cot_hi = [list(range(int(off_hi[t]), int(off_hi[t + 1]))) for t in range(NT)]
    return in_maps, Ctot, Clo_tot, cot_lo, cot_hi


_WAIT_LIMIT = int(os.environ.get("GCN_WAIT_LIMIT", "1"))


def _legalize_waits(nc, limit=None):
    """Split >limit semaphore waits into standalone NOPs on the same engine.

    Walrus codegen rejects instructions whose sync_info carries more wait
    conditions than the ISA sync fields hold ("Too many sync wait commands").
    A chain of no-ops each carrying <=limit waits is semantically identical
    (waits are AND conditions and the engine queue is in-order).
    """
    if limit is None:
        limit = _WAIT_LIMIT
    import bass_rust as _br

    uid = 0
    for fn in nc.m.functions:
        for bb in fn.blocks:
            out = []
            changed = False
            for ins in bb.instructions:
                si = ins.sync_info
                if si is not None and len(si.on_wait) > limit:
                    waits = list(si.on_wait)
                    excess, keep = waits[:-limit], waits[-limit:]
                    for i in range(0, len(excess), limit):
                        nop = mybir.InstNoOp(name=f"waitsplit_{uid}", ins=[], outs=[])
                        uid += 1
                        nop.engine = ins.engine
                        nop.sync_info = _br.SyncInfo(
                            on_wait=excess[i : i + limit], on_update=[]
                        )
                        out.append(nop)
                    ins.sync_info = _br.SyncInfo(
                        on_wait=keep, on_update=list(si.on_update)
                    )
                    changed = True
                out.append(ins)
            if changed:
                bb.instructions = out


def _build(Ctot, Clo_tot, cot_lo, cot_hi):
    dt = mybir.dt
    Alu = mybir.AluOpType
    Act = mybir.ActivationFunctionType

    nc = bass.Bass(num_devices=NCORES)

    # ---- I/O ----
    xpack_e = nc.dram_tensor("xpack", [P, XCOLS], dt.bfloat16, kind="ExternalInput")
    w1_e = nc.dram_tensor("w1", [2 * F, F], dt.bfloat16, kind="ExternalInput")
    w2_e = nc.dram_tensor("w2", [F, FO], dt.bfloat16, kind="ExternalInput")
    b1_e = nc.dram_tensor("b1r", [P, F], dt.float32, kind="ExternalInput")
    b2_e = nc.dram_tensor("b2r", [P, FO], dt.float32, kind="ExternalInput")
    dinvh_e = nc.dram_tensor("dinvh", [P, NTH], dt.float32, kind="ExternalInput")
    dinvs_e = nc.dram_tensor("dinvs", [P, NT], dt.float32, kind="ExternalInput")
    idx16_e = nc.dram_tensor("idx16", [P, Ctot * 8], dt.int16, kind="ExternalInput")
    selv_e = nc.dram_tensor("selv", [P, Ctot], dt.float32, kind="ExternalInput")
    out_e = nc.dram_tensor("out", [P, NT * FO], dt.float32, kind="ExternalOutput")
    debug = bool(int(os.environ.get("GCN_DEBUG", "0")))
    phases = int(os.environ.get("GCN_PHASES", "3"))
    if debug:
        dbg1_e = nc.dram_tensor(
            "dbg_out1", [P, NT * F], dt.bfloat16, kind="ExternalOutput"
        )
        dbgg_e = nc.dram_tensor(
            "dbg_g0", [P, G * EPR], dt.bfloat16, kind="ExternalOutput"
        )

    # ---- internal DRAM ----
    # padded-row layouts: node n -> row (n%128)*NTH + n//128, 256B rows,
    # only the leading F (layer 1) / FO (layer 2) columns are meaningful.
    h_dram = nc.dram_tensor("h_dram", [NPAD, EPR], dt.bfloat16)
    h2_dram = nc.dram_tensor("h2_dram", [NPAD, EPR], dt.bfloat16)
    cc_in = nc.dram_tensor("cc_in", [P, NT * FO], dt.bfloat16)
    cc_out = nc.dram_tensor(
        "cc_out", [NCORES, P, NT * FO], dt.bfloat16, addr_space="Shared"
    )

    NGR_LO = (Clo_tot + G - 1) // G
    NGR_HI = (Ctot - Clo_tot + G - 1) // G

    def chunk_rhs(gbufs, j, base_chunk, fw):
        g, jl = (j - base_chunk) // G, (j - base_chunk) % G
        return gbufs[g][:, jl * EPR : jl * EPR + fw]

    with tile.TileContext(nc) as tc:
        with tc.tile_pool(name="const", bufs=1) as cp:
            w1 = cp.tile([2 * F, F], dt.bfloat16, tag="w1")
            nc.sync.dma_start(out=w1[:], in_=w1_e[:, :])
            w2 = cp.tile([F, FO], dt.bfloat16, tag="w2")
            nc.sync.dma_start(out=w2[:], in_=w2_e[:, :])
            b1r = cp.tile([P, F], dt.float32, tag="b1r")
            nc.sync.dma_start(out=b1r[:], in_=b1_e[:, :])
            b2r = cp.tile([P, FO], dt.float32, tag="b2r")
            nc.sync.dma_start(out=b2r[:], in_=b2_e[:, :])
            dinvh = cp.tile([P, NTH], dt.float32, tag="dinvh")
            nc.sync.dma_start(out=dinvh[:], in_=dinvh_e[:, :])
            dinvs = cp.tile([P, NT], dt.float32, tag="dinvs")
            nc.sync.dma_start(out=dinvs[:], in_=dinvs_e[:, :])
            idx16 = cp.tile([P, Ctot * 8], dt.int16, tag="idx16")
            nc.sync.dma_start(out=idx16[:], in_=idx16_e[:, :])
            selv = cp.tile([P, Ctot], dt.float32, tag="selv")
            nc.sync.dma_start(out=selv[:], in_=selv_e[:, :])
            iota = cp.tile([P, P], dt.bfloat16, tag="iota")
            nc.gpsimd.iota(
                iota[:],
                pattern=[[1, P]],
                base=0,
                channel_multiplier=0,
                allow_small_or_imprecise_dtypes=True,
            )
            ident = cp.tile([P, P], dt.bfloat16, tag="ident")
            make_identity(nc, ident[:])
            out1 = cp.tile([P, NT * F], dt.bfloat16, tag="out1")
            acc1 = cp.tile([P, NT * F], dt.float32, tag="acc1")
            h2st = cp.tile([P, NT * FO], dt.bfloat16, tag="h2st")
            outst = cp.tile([P, NT * FO], dt.float32, tag="outst")

            tc.strict_bb_all_engine_barrier()
            # dma_gather lives in the Q7 "mlp" extended-instruction library.
            # bass's pseudo reload ships with an empty instr payload, which
            # walrus rejects ("ISA wrong length") — fill the 64B struct.
            _li = nc.gpsimd.load_library(library_config.mlp)
            _instr, _fx = bass_isa.isa_struct(
                nc.isa,
                nc.isa.Opcode.NEURON_ISA_TPB_OPCODE_PSEUDO_INST,
                {"pseudo_opcode": 2, "lib_index": library_config.mlp.index},
                struct_name="NEURON_ISA_TPB_PSEUDO_LIBRARY_RELOAD_INDEX_STRUCT",
            )
            _li.ins.instr = _instr

            # ---------------- Phase A: h' = bf16(dinv * (x @ W1)) ------------
            XB = 1792  # 14 node tiles per (block, half); 14 blocks
            with (
                tc.tile_pool(name="xa", bufs=3) as xpool,
                tc.tile_pool(name="ha", bufs=3) as hpool,
                tc.tile_pool(name="pha", bufs=4, space="PSUM") as phpool,
            ):
                for blk in range(0, XCOLS, XB):
                    xb = xpool.tile([P, XB], dt.bfloat16, tag="xb")
                    nc.sync.dma_start(out=xb[:], in_=xpack_e[:, blk : blk + XB])
                    for a in (0, 1):
                        nt_blk = XB // P
                        hst = hpool.tile([P, nt_blk * F], dt.bfloat16, tag="hst")
                        for m in range(nt_blk):
                            tg = a * XH + blk // P + m
                            ph = phpool.tile([P, F], dt.float32, tag="ph")
                            nc.tensor.matmul(
                                out=ph[:],
                                lhsT=xb[a * F : (a + 1) * F, m * P : (m + 1) * P],
                                rhs=w1[a * F : (a + 1) * F, :],
                                start=True,
                                stop=True,
                            )
                            nc.scalar.activation(
                                out=hst[:, m * F : (m + 1) * F],
                                in_=ph[:],
                                func=Act.Copy,
                                scale=dinvh[:, tg : tg + 1],
                            )
                        tg0 = a * XH + blk // P
                        # dest rows r = p*NTH + tg, tg in [tg0, tg0+nt_blk),
                        # first F cols; src order (p, m, f)
                        dst = bass.AP(
                            h_dram,
                            tg0 * EPR,
                            [[NTH * EPR, P], [EPR, nt_blk], [1, F]],
                        )
                        nc.sync.dma_start(out=dst, in_=hst[:])


            # ---------------- aggregation machinery --------------------------
            def aggregation(src_dram, fw, epilogue, dump_g0=False):
                """Two-pass (low rows / high rows) chunked aggregation.

                Low pass: psum -> acc1 (f32).  High pass: psum; epilogue(t, pt)
                consumes pt (high psum) + acc1 slice (low partial).
                """
                src_lo = src_dram.ap()  # [NPAD, EPR]
                src_hi = bass.AP(
                    src_dram,
                    LOWROWS * EPR,
                    [[EPR, NPAD - LOWROWS], [1, EPR]],
                )
                with (
                    tc.tile_pool(name="gb", bufs=4) as gpool,
                    tc.tile_pool(name="selp", bufs=10) as selpool,
                    tc.tile_pool(name="paggp", bufs=8, space="PSUM") as paggpool,
                ):
                    gbufs_lo = [None] * NGR_LO
                    gbufs_hi = [None] * NGR_HI

                    def issue(g, base_chunk, nchunks, srcv, store):
                        c0 = base_chunk + g * G
                        w = min(nchunks, (g + 1) * G) - g * G
                        gb = gpool.tile([P, G * EPR], dt.bfloat16, tag="gb")
                        nc.gpsimd.dma_gather(
                            out_ap=gb[:, : w * EPR].rearrange(
                                "p (s e) -> p s e", e=EPR
                            ),
                            in_ap=srcv,
                            idxs_ap=idx16[:, c0 * 8 : (c0 + w) * 8],
                            num_idxs=w * P,
                            num_idxs_reg=w * P,
                            elem_size=EPR,
                            single_packet=False,
                        )
                        store[g] = gb
                        if dump_g0 and base_chunk == 0 and g == 0:
                            nc.sync.dma_start(out=dbgg_e[:, :], in_=gb[:])

                    def build_sel(j):
                        sel = selpool.tile([P, P], dt.bfloat16, tag="sel")
                        nc.vector.tensor_scalar(
                            out=sel[:],
                            in0=iota[:],
                            scalar1=selv[:, j : j + 1],
                            scalar2=None,
                            op0=Alu.is_equal,
                        )
                        return sel

                    # ---- low pass: accumulate into acc1 ----
                    issued = 0
                    for t in range(NT):
                        if not cot_lo[t]:
                            nc.vector.memset(acc1[:, t * F : t * F + fw], 0.0)
                            continue
                        while issued * G < cot_lo[t][-1] + 1:
                            issue(issued, 0, Clo_tot, src_lo, gbufs_lo)
                            issued += 1
                        pt = paggpool.tile([P, F], dt.float32, tag="pagg")
                        for k, j in enumerate(cot_lo[t]):
                            sel = build_sel(j)
                            nc.tensor.matmul(
                                out=pt[:, :fw],
                                lhsT=sel[:],
                                rhs=chunk_rhs(gbufs_lo, j, 0, fw),
                                start=(k == 0),
                                stop=(k == len(cot_lo[t]) - 1),
                            )
                        nc.vector.tensor_copy(
                            out=acc1[:, t * F : t * F + fw], in_=pt[:, :fw]
                        )
                    # ---- high pass: psum + epilogue ----
                    issued = 0
                    for t in range(NT):
                        pt = paggpool.tile([P, F], dt.float32, tag="pagg")
                        if cot_hi[t]:
                            while issued * G < cot_hi[t][-1] - Clo_tot + 1:
                                issue(issued, Clo_tot, Ctot - Clo_tot, src_hi,
                                      gbufs_hi)
                                issued += 1
                            for k, j in enumerate(cot_hi[t]):
                                sel = build_sel(j)
                                nc.tensor.matmul(
                                    out=pt[:, :fw],
                                    lhsT=sel[:],
                                    rhs=chunk_rhs(gbufs_hi, j, Clo_tot, fw),
                                    start=(k == 0),
                                    stop=(k == len(cot_hi[t]) - 1),
                                )
                        else:
                            nc.vector.memset(pt[:, :fw], 0.0)
                        epilogue(t, pt)

            # ---------------- Phase B: layer-1 aggregation -------------------
            if phases >= 1:
              with tc.tile_pool(name="ep1", bufs=4) as ep1pool:

                def epi1(t, pt):
                    tmp = ep1pool.tile([P, F], dt.float32, tag="tmp")
                    nc.vector.tensor_tensor(
                        out=tmp[:],
                        in0=pt[:, :F],
                        in1=acc1[:, t * F : (t + 1) * F],
                        op=Alu.add,
                    )
                    nc.vector.tensor_scalar(
                        out=tmp[:],
                        in0=tmp[:],
                        scalar1=dinvs[:, t : t + 1],
                        scalar2=None,
                        op0=Alu.mult,
                    )
                    nc.vector.tensor_tensor(
                        out=tmp[:], in0=tmp[:], in1=b1r[:], op=Alu.add
                    )
                    nc.scalar.activation(
                        out=out1[:, t * F : (t + 1) * F], in_=tmp[:], func=Act.Relu
                    )

                aggregation(h_dram, F, epi1, dump_g0=debug)

            if debug:
                nc.sync.dma_start(out=dbg1_e[:, :], in_=out1[:])

            # ---------------- Phase C: h2' = bf16(dinv * (out1 @ W2)) --------
            if phases >= 2:
              with (
                tc.tile_pool(name="ptrp", bufs=2, space="PSUM") as ptrpool,
                tc.tile_pool(name="ph2p", bufs=2, space="PSUM") as ph2pool,
                tc.tile_pool(name="o1tp", bufs=2) as o1tpool,
            ):
                for t in range(NT):
                    ptr_ = ptrpool.tile([P, P], dt.bfloat16, tag="ptr")
                    nc.tensor.transpose(
                        out=ptr_[:F, :],
                        in_=out1[:, t * F : (t + 1) * F],
                        identity=ident[:],
                    )
                    o1T = o1tpool.tile([F, P], dt.bfloat16, tag="o1T")
                    nc.vector.tensor_copy(out=o1T[:], in_=ptr_[:F, :])
                    ph2 = ph2pool.tile([P, FO], dt.float32, tag="ph2")
                    nc.tensor.matmul(
                        out=ph2[:], lhsT=o1T[:], rhs=w2[:, :], start=True, stop=True
                    )
                    nc.scalar.activation(
                        out=h2st[:, t * FO : (t + 1) * FO],
                        in_=ph2[:],
                        func=Act.Copy,
                        scale=dinvs[:, t : t + 1],
                    )
                nc.sync.dma_start(out=cc_in[:, :], in_=h2st[:])

            if phases >= 2:
              nc.gpsimd.collective_compute(
                "AllGather",
                mybir.AluOpType.bypass,
                replica_groups=[list(range(NCORES))],
                ins=[cc_in.ap()],
                outs=[cc_out.ap()],
            )

            if phases >= 2:

              # expand cc_out [c, p, (t f)] -> h2_dram rows (p*NTH + c*NT + t),
            # first FO cols.  src order (c, p, t, f); dest AP same order.
              with tc.tile_pool(name="exp", bufs=1) as expool:
                xt = expool.tile([P, NCORES * NT * FO], dt.bfloat16, tag="xt")
                for c in range(NCORES):
                    nc.sync.dma_start(
                        out=xt[:, c * NT * FO : (c + 1) * NT * FO],
                        in_=cc_out[c],
                    )
                dst = bass.AP(
                    h2_dram,
                    0,
                    [[NTH * EPR, P], [NT * EPR, NCORES], [EPR, NT], [1, FO]],
                )
                nc.sync.dma_start(out=dst, in_=xt[:])


            # ---------------- Phase D: layer-2 aggregation + log_softmax -----
            if phases >= 3:
              with tc.tile_pool(name="ep2", bufs=4) as ep2pool:

                def epi2(t, pt):
                    tmp = ep2pool.tile([P, FO], dt.float32, tag="tmp2")
                    nc.vector.tensor_tensor(
                        out=tmp[:],
                        in0=pt[:, :FO],
                        in1=acc1[:, t * F : t * F + FO],
                        op=Alu.add,
                    )
                    nc.vector.tensor_scalar(
                        out=tmp[:],
                        in0=tmp[:],
                        scalar1=dinvs[:, t : t + 1],
                        scalar2=None,
                        op0=Alu.mult,
                    )
                    nc.vector.tensor_tensor(
                        out=tmp[:], in0=tmp[:], in1=b2r[:], op=Alu.add
                    )
                    mx = ep2pool.tile([P, 1], dt.float32, tag="mx")
                    nc.vector.reduce_max(
                        out=mx[:], in_=tmp[:], axis=mybir.AxisListType.X, negate=True
                    )
                    ex = ep2pool.tile([P, FO], dt.float32, tag="ex")
                    nc.scalar.activation(
                        out=ex[:], in_=tmp[:], func=Act.Exp, bias=mx[:, 0:1]
                    )
                    sm = ep2pool.tile([P, 1], dt.float32, tag="sm")
                    nc.vector.reduce_sum(
                        out=sm[:], in_=ex[:], axis=mybir.AxisListType.X
                    )
                    lg = ep2pool.tile([P, 1], dt.float32, tag="lg")
                    nc.scalar.activation(out=lg[:], in_=sm[:], func=Act.Ln)
                    nc.vector.tensor_scalar(
                        out=outst[:, t * FO : (t + 1) * FO],
                        in0=tmp[:],
                        scalar1=mx[:, 0:1],
                        scalar2=lg[:, 0:1],
                        op0=Alu.add,
                        op1=Alu.subtract,
                    )

                aggregation(h2_dram, FO, epi2)
            if phases >= 3:
                nc.sync.dma_start(out=out_e[:, :], in_=outst[:])
            else:
                nc.vector.memset(outst[:], 0.0)
                nc.sync.dma_start(out=out_e[:, :], in_=outst[:])

    _legalize_waits(nc)
    return nc


def kernel(x, edge_index, W1, b1, W2, b2, _trace=False, _trace_kwargs=None):
    in_maps, Ctot, Clo_tot, cot_lo, cot_hi = _preprocess(
        x, edge_index, W1, b1, W2, b2
    )
    key = (
        Ctot,
        Clo_tot,
        tuple(len(c) for c in cot_lo),
        tuple(len(c) for c in cot_hi),
    )
    if key not in _CACHE:
        _CACHE[key] = _build(Ctot, Clo_tot, cot_lo, cot_hi)
    nc = _CACHE[key]

    res = run_bass_kernel_spmd(
        nc,
        in_maps,
        core_ids=list(range(NCORES)),
        trace=_trace,
        **(_trace_kwargs or {}),
    )
    out = np.empty((N, FO), dtype=np.float32)
    for c in range(NCORES):
        o = np.asarray(res.results[c]["out"], dtype=np.float32)
        o = o.reshape(P, NT, FO).transpose(1, 0, 2).reshape(NT * P, FO)
        k = min(SH, N - c * SH)
        out[c * SH : c * SH + k] = o[:k]
    kernel._last_result = res
    return out



# revision 3
# speedup vs baseline: 1.7384x; 1.7384x over previous
"""GCN (2-layer, PyG GCNConv-style) on 8 Trainium2 NeuronCores.

v2 strategy — degree-sorted identity aggregation:
  - Nodes globally sorted by in-degree (random edges), padded to 50176
    positions; 128-position blocks dealt round-robin to cores (block b ->
    core b%8, tile b//8).  A dst tile therefore holds 128 near-equal-degree
    nodes, so per-tile "rounds" (one edge per dst lane per round) pad
    almost nothing:  rounds_t = 1 + max in-degree over the 8 sibling
    blocks, with a trailing self-loop round.
  - Aggregation is a per-round dma_gather of the 128 lanes' source rows
    (slot == lane) followed by an identity matmul accumulating into the
    tile's PSUM — no per-chunk selection-matrix builds at all.
  - The node table h_dram has one 256B row per position: bytes [0:128) =
    layer-1 features (64 bf16), bytes [128:160) = layer-2 features
    (16 bf16, written between layers).  Both layers share one int16 index
    table; the gather base is biased to row 32768 so signed indices cover
    all 50176 rows.  Gathers use raw InstDMAGatherAnt with elem_size 64
    (layer 1) / 16 (layer 2) elements and elem_step 128 (256B stride).
  - Pad slots (lanes whose degree < round count) and per-instruction
    flush chunks fetch the all-zero last pad row, keeping every gather
    instruction's trailing index non-negative (Q7 trims trailing
    negatives).
  - Phase A computes h1' = (dinv*x) @ W1 (x pre-scaled on host), batching
    7 tiles per PSUM bank; epilogues fold dinv_dst via activation scale;
    log_softmax on-chip; host un-permutes the output.
"""

import os
import sys

import numpy as np

for _p in ("/opt/trn_rl_repo", "/root/.axon_site/_ro/trn_rl_repo"):
    if os.path.isdir(_p) and _p not in sys.path:
        sys.path.insert(0, _p)

import ml_dtypes  # noqa: E402
import concourse.bass as bass  # noqa: E402
import concourse.mybir as mybir  # noqa: E402
import concourse.tile as tile  # noqa: E402
from concourse.bass_utils import run_bass_kernel_spmd  # noqa: E402
from concourse.masks import make_identity  # noqa: E402
from concourse import library_config  # noqa: E402
import concourse.bass_isa as bass_isa  # noqa: E402

# ---------------- static problem config (hardcoded per contract) -------------
N = 50000
E = 800000
F = 64          # F_IN == F_HID
FO = 16         # F_OUT
NCORES = 8
P = 128
NBLK = 392                # 128-position blocks
NPAD = NBLK * P           # 50176 positions
NT = NBLK // NCORES       # 49 tiles per core
ROWE = 128                # table row length in bf16 elements (256B stride)
BIAS = 32768              # gather base row (signed int16 indices)
PADPOS = NPAD - 1         # all-zero pad row
GI = 64                   # gather chunks per instruction
XCOLS = NPAD // 2         # 25088 columns per xpack half

BF16 = ml_dtypes.bfloat16

_CACHE = {}


def _pack_idx16(vals):
    """Slot-ordered int16 values [C*128] -> idx table [128, C*8].

    dma_gather reads index k from (partition k%16, col k//16), replicated
    across the 8 q7 cores (partition stripes of 16).
    """
    k = np.arange(vals.size)
    tbl = np.zeros((16, vals.size // 16), dtype=np.int16)
    tbl[k % 16, k // 16] = vals.astype(np.int16)
    return np.tile(tbl, (8, 1))


def _preprocess(x, edge_index, W1, b1, W2, b2):
    src = np.asarray(edge_index[0], dtype=np.int64)
    dst = np.asarray(edge_index[1], dtype=np.int64)

    rdeg = np.bincount(dst, minlength=N)
    dinv = (1.0 / np.sqrt(rdeg + 1.0)).astype(np.float32)

    order = np.argsort(-rdeg, kind="stable")          # node at each position
    norder = np.concatenate([order, np.full(NPAD - N, -1, dtype=np.int64)])
    pos = np.empty(N, dtype=np.int64)
    pos[order] = np.arange(N)

    posdinv = np.zeros(NPAD, dtype=np.float32)
    posdinv[pos] = dinv
    prdeg = np.zeros(NPAD, dtype=np.int64)
    prdeg[pos] = rdeg

    # per-position incoming-edge source lists (by position ids)
    pd = pos[dst]
    ps = pos[src]
    eorder = np.argsort(pd, kind="stable")
    ps_s = ps[eorder]
    starts = np.searchsorted(pd[eorder], np.arange(NPAD + 1))

    R = [1 + int(prdeg[1024 * t : 1024 * (t + 1)].max()) for t in range(NT)]

    # chunk stream structure (uniform across cores): (tile, round | -1=flush)
    chunks = []
    n = 0
    for t in range(NT):
        for r in range(R[t]):
            chunks.append((t, r))
            n += 1
            last = t == NT - 1 and r == R[t] - 1
            if n % GI == GI - 1 and not last:
                chunks.append((t, -1))
                n += 1
    Ctot = len(chunks)

    # per-core slot values (position ids; self round last)
    lanes = np.arange(P)
    idx16 = np.empty((NCORES, P, Ctot * 8), dtype=np.int16)
    for c in range(NCORES):
        vals = np.empty((Ctot, P), dtype=np.int64)
        for k, (t, r) in enumerate(chunks):
            if r < 0:
                vals[k] = PADPOS
                continue
            q = 1024 * t + P * c + lanes
            if r == R[t] - 1:
                vals[k] = q                      # self loop
            else:
                cnt = prdeg[q]
                v = np.full(P, PADPOS, dtype=np.int64)
                m = r < cnt
                v[m] = ps_s[starts[q[m]] + r]
                vals[k] = v
        idx16[c] = _pack_idx16((vals - BIAS).ravel())

    # xpack: x pre-scaled by dinv, transposed, two halves stacked on partitions
    xp = np.zeros((NPAD, F), dtype=np.float32)
    real = norder >= 0
    xp[real] = np.asarray(x, dtype=np.float32)[norder[real]] * posdinv[real, None]
    xpack = xp.reshape(2, XCOLS, F).transpose(0, 2, 1).reshape(P, XCOLS).astype(BF16)

    dinvs = np.zeros((NCORES, P, NT), dtype=np.float32)
    for c in range(NCORES):
        for t in range(NT):
            dinvs[c, :, t] = posdinv[1024 * t + P * c + lanes]

    common = {
        "xpack": xpack,
        "w1": np.concatenate([np.asarray(W1, np.float32)] * 2, axis=0).astype(BF16),
        "w2": np.asarray(W2, dtype=np.float32).astype(BF16),
        "b1r": np.broadcast_to(np.asarray(b1, np.float32), (P, F)).copy(),
        "b2r": np.broadcast_to(np.asarray(b2, np.float32), (P, FO)).copy(),
    }
    in_maps = []
    for c in range(NCORES):
        m = dict(common)
        m["dinvs"] = dinvs[c]
        m["idx16"] = idx16[c]
        in_maps.append(m)
    return in_maps, chunks, norder


_WAIT_LIMIT = int(os.environ.get("GCN_WAIT_LIMIT", "1"))


def _legalize_waits(nc, limit=None):
    """Split >limit semaphore waits into standalone NOPs on the same engine.

    Walrus codegen rejects instructions whose sync_info carries more wait
    conditions than the ISA sync fields hold ("Too many sync wait commands").
    A chain of no-ops each carrying <=limit waits is semantically identical
    (waits are AND conditions and the engine queue is in-order).
    """
    if limit is None:
        limit = _WAIT_LIMIT
    import bass_rust as _br

    uid = 0
    for fn in nc.m.functions:
        for bb in fn.blocks:
            out = []
            changed = False
            for ins in bb.instructions:
                si = ins.sync_info
                if si is not None and len(si.on_wait) > limit:
                    waits = list(si.on_wait)
                    excess, keep = waits[:-limit], waits[-limit:]
                    for i in range(0, len(excess), limit):
                        nop = mybir.InstNoOp(name=f"waitsplit_{uid}", ins=[], outs=[])
                        uid += 1
                        nop.engine = ins.engine
                        nop.sync_info = _br.SyncInfo(
                            on_wait=excess[i : i + limit], on_update=[]
                        )
                        out.append(nop)
                    ins.sync_info = _br.SyncInfo(
                        on_wait=keep, on_update=list(si.on_update)
                    )
                    changed = True
                out.append(ins)
            if changed:
                bb.instructions = out


def _dma_gather_raw(nc, out_ap, in_ap, idxs_ap, num_idxs, elem_size, elem_step):
    """dma_gather with elem_size not a multiple of 256B (bass.py over-asserts
    the transpose-path alignment).  Mirrors the tail of BassGpSimd.dma_gather
    for the DRAM-source, transpose=False case: per-index descriptors read
    elem_size elements from base + idx*elem_step (stride must be 256B-aligned,
    elem_size is free)."""
    eng = nc.gpsimd
    stride_bytes = elem_step * mybir.dt.size(in_ap.dtype)
    assert stride_bytes % 256 == 0
    _in_ap = eng.lower_ap_dma(in_ap, for_custom_bir_dma=True)
    _idxs_ap = eng.lower_ap(idxs_ap)
    _out_ap = eng.lower_ap(out_ap)
    return eng.add_instruction(
        mybir.InstDMAGatherAnt(
            name=nc.get_next_instruction_name(),
            ins=[*_in_ap, _idxs_ap, eng.lower_val_access(eng.to_reg(num_idxs))],
            outs=[_out_ap],
            transpose=False,
            num_idxs=num_idxs,
            elem_size=elem_size,
            stride_bytes_256=stride_bytes // 256,
            gen_mode=0,
            single_packet=False,
            queue_num=0,
            sbuf_tokens_per_rank=0,
            sbuf_free_dim_per_rank=0,
            sbuf_free_dim_pad_per_rank=0,
            sbuf_byte_offset=0,
        )
    )


def _build(chunks):
    dt = mybir.dt
    Alu = mybir.AluOpType
    Act = mybir.ActivationFunctionType

    Ctot = len(chunks)
    # gather instruction groups [c0, c1)
    groups = [(g, min(g + GI, Ctot)) for g in range(0, Ctot, GI)]
    # per-tile chunk index lists
    tchunks = [[] for _ in range(NT)]
    for k, (t, _r) in enumerate(chunks):
        tchunks[t].append(k)

    nc = bass.Bass(num_devices=NCORES)

    xpack_e = nc.dram_tensor("xpack", [P, XCOLS], dt.bfloat16, kind="ExternalInput")
    w1_e = nc.dram_tensor("w1", [2 * F, F], dt.bfloat16, kind="ExternalInput")
    w2_e = nc.dram_tensor("w2", [F, FO], dt.bfloat16, kind="ExternalInput")
    b1_e = nc.dram_tensor("b1r", [P, F], dt.float32, kind="ExternalInput")
    b2_e = nc.dram_tensor("b2r", [P, FO], dt.float32, kind="ExternalInput")
    dinvs_e = nc.dram_tensor("dinvs", [P, NT], dt.float32, kind="ExternalInput")
    idx16_e = nc.dram_tensor("idx16", [P, Ctot * 8], dt.int16, kind="ExternalInput")
    out_e = nc.dram_tensor("out", [P, NT * FO], dt.float32, kind="ExternalOutput")
    phases = int(os.environ.get("GCN_PHASES", "3"))

    h_dram = nc.dram_tensor("h_dram", [NPAD, ROWE], dt.bfloat16)
    cc_in = nc.dram_tensor("cc_in", [P, NT * FO], dt.bfloat16)
    cc_out = nc.dram_tensor(
        "cc_out", [NCORES, P, NT * FO], dt.bfloat16, addr_space="Shared"
    )

    with tile.TileContext(nc) as tc:
        with tc.tile_pool(name="const", bufs=1) as cp:
            w1 = cp.tile([2 * F, F], dt.bfloat16, tag="w1")
            nc.sync.dma_start(out=w1[:], in_=w1_e[:, :])
            w2 = cp.tile([F, FO], dt.bfloat16, tag="w2")
            nc.sync.dma_start(out=w2[:], in_=w2_e[:, :])
            b1r = cp.tile([P, F], dt.float32, tag="b1r")
            nc.sync.dma_start(out=b1r[:], in_=b1_e[:, :])
            b2r = cp.tile([P, FO], dt.float32, tag="b2r")
            nc.sync.dma_start(out=b2r[:], in_=b2_e[:, :])
            dinvs = cp.tile([P, NT], dt.float32, tag="dinvs")
            nc.sync.dma_start(out=dinvs[:], in_=dinvs_e[:, :])
            idx16 = cp.tile([P, Ctot * 8], dt.int16, tag="idx16")
            nc.sync.dma_start(out=idx16[:], in_=idx16_e[:, :])
            ident = cp.tile([P, P], dt.bfloat16, tag="ident")
            make_identity(nc, ident[:])
            out1 = cp.tile([P, NT * F], dt.bfloat16, tag="out1")
            h2st = cp.tile([P, NT * FO], dt.bfloat16, tag="h2st")
            outst = cp.tile([P, NT * FO], dt.float32, tag="outst")

            tc.strict_bb_all_engine_barrier()
            # dma_gather lives in the Q7 "mlp" extended-instruction library.
            # bass's pseudo reload ships with an empty instr payload, which
            # walrus rejects ("ISA wrong length") — fill the 64B struct.
            _li = nc.gpsimd.load_library(library_config.mlp)
            _instr, _fx = bass_isa.isa_struct(
                nc.isa,
                nc.isa.Opcode.NEURON_ISA_TPB_OPCODE_PSEUDO_INST,
                {"pseudo_opcode": 2, "lib_index": library_config.mlp.index},
                struct_name="NEURON_ISA_TPB_PSEUDO_LIBRARY_RELOAD_INDEX_STRUCT",
            )
            _li.ins.instr = _instr

            # ---------------- Phase A: h1' = (dinv*x) @ W1 -> table ----------
            XB = 1792  # 14 tiles per (block, half); 14 blocks
            TB = 7     # tiles batched per PSUM bank
            with (
                tc.tile_pool(name="xa", bufs=3) as xpool,
                tc.tile_pool(name="ha", bufs=3) as hpool,
                tc.tile_pool(name="pha", bufs=3, space="PSUM") as phpool,
            ):
                for blk in range(0, XCOLS, XB):
                    xb = xpool.tile([P, XB], dt.bfloat16, tag="xb")
                    nc.sync.dma_start(out=xb[:], in_=xpack_e[:, blk : blk + XB])
                    for a in (0, 1):
                        for u0 in range(0, XB // P, TB):
                            ph = phpool.tile([P, TB * F], dt.float32, tag="ph")
                            for m in range(TB):
                                mm = u0 + m
                                nc.tensor.matmul(
                                    out=ph[:, m * F : (m + 1) * F],
                                    lhsT=xb[a * F : (a + 1) * F,
                                            mm * P : (mm + 1) * P],
                                    rhs=w1[a * F : (a + 1) * F, :],
                                    start=True,
                                    stop=True,
                                )
                            hst = hpool.tile([P, TB * F], dt.bfloat16, tag="hst")
                            nc.scalar.activation(out=hst[:], in_=ph[:], func=Act.Copy)
                            g0 = a * (XCOLS // P) + blk // P + u0
                            dst = bass.AP(
                                h_dram,
                                g0 * P * ROWE,
                                [[ROWE, P], [P * ROWE, TB], [1, F]],
                            )
                            nc.sync.dma_start(out=dst, in_=hst[:])

            tc.strict_bb_all_engine_barrier()

            # ---------------- aggregation machinery --------------------------
            def aggregation(elem, col0, fw, epilogue):
                """Identity-matmul rounds over the shared chunk stream."""
                src = bass.AP(
                    h_dram,
                    BIAS * ROWE + col0,
                    [[ROWE, NPAD - BIAS], [1, elem]],
                )
                with (
                    tc.tile_pool(name="gb", bufs=3) as gpool,
                    tc.tile_pool(name="pagg", bufs=4, space="PSUM") as ppool,
                ):
                    gbufs = [None] * len(groups)

                    def issue(g):
                        c0, c1 = groups[g]
                        w = c1 - c0
                        gb = gpool.tile([P, GI * elem], dt.bfloat16, tag="gb")
                        _dma_gather_raw(
                            nc,
                            out_ap=gb[:, : w * elem].rearrange(
                                "p (s e) -> p s e", e=elem
                            ),
                            in_ap=src,
                            idxs_ap=idx16[:, c0 * 8 : c1 * 8],
                            num_idxs=w * P,
                            elem_size=elem,
                            elem_step=ROWE,
                        )
                        gbufs[g] = gb

                    issued = 0
                    for t in range(NT):
                        ks = tchunks[t]
                        while issued * GI < ks[-1] + 1:
                            issue(issued)
                            issued += 1
                        pt = ppool.tile([P, fw], dt.float32, tag="pt")
                        for i, k in enumerate(ks):
                            g, kl = k // GI, k % GI
                            nc.tensor.matmul(
                                out=pt[:],
                                lhsT=ident[:],
                                rhs=gbufs[g][:, kl * elem : kl * elem + fw],
                                start=(i == 0),
                                stop=(i == len(ks) - 1),
                            )
                        epilogue(t, pt)

            # ---------------- Phase B: layer-1 aggregation -------------------
            if phases >= 1:
                with tc.tile_pool(name="ep1", bufs=4) as ep1pool:

                    def epi1(t, pt):
                        tmp = ep1pool.tile([P, F], dt.float32, tag="tmp")
                        nc.scalar.activation(
                            out=tmp[:],
                            in_=pt[:],
                            func=Act.Copy,
                            scale=dinvs[:, t : t + 1],
                        )
                        nc.vector.tensor_tensor(
                            out=tmp[:], in0=tmp[:], in1=b1r[:], op=Alu.add
                        )
                        nc.scalar.activation(
                            out=out1[:, t * F : (t + 1) * F], in_=tmp[:],
                            func=Act.Relu,
                        )

                    aggregation(F, 0, F, epi1)

            # ---------------- Phase C: h2' = dinv * (out1 @ W2) -> table -----
            if phases >= 2:
                with (
                    tc.tile_pool(name="ptrp", bufs=2, space="PSUM") as ptrpool,
                    tc.tile_pool(name="ph2p", bufs=2, space="PSUM") as ph2pool,
                    tc.tile_pool(name="o1tp", bufs=2) as o1tpool,
                ):
                    for t in range(NT):
                        ptr_ = ptrpool.tile([P, P], dt.bfloat16, tag="ptr")
                        nc.tensor.transpose(
                            out=ptr_[:F, :],
                            in_=out1[:, t * F : (t + 1) * F],
                            identity=ident[:],
                        )
                        o1T = o1tpool.tile([F, P], dt.bfloat16, tag="o1T")
                        nc.vector.tensor_copy(out=o1T[:], in_=ptr_[:F, :])
                        ph2 = ph2pool.tile([P, FO], dt.float32, tag="ph2")
                        nc.tensor.matmul(
                            out=ph2[:], lhsT=o1T[:], rhs=w2[:, :],
                            start=True, stop=True,
                        )
                        nc.scalar.activation(
                            out=h2st[:, t * FO : (t + 1) * FO],
                            in_=ph2[:],
                            func=Act.Copy,
                            scale=dinvs[:, t : t + 1],
                        )
                    nc.sync.dma_start(out=cc_in[:, :], in_=h2st[:])

            if phases >= 2 and not int(os.environ.get("GCN_NO_CC", "0")):
                nc.gpsimd.collective_compute(
                    "AllGather",
                    mybir.AluOpType.bypass,
                    replica_groups=[list(range(NCORES))],
                    ins=[cc_in.ap()],
                    outs=[cc_out.ap()],
                )

                # scatter h2 into table rows: position q = 1024t + 128c + j,
                # row q elements [64:80)
                for c in range(NCORES):
                    dst = bass.AP(
                        h_dram,
                        c * P * ROWE + F,
                        [[ROWE, P], [1024 * ROWE, NT], [1, FO]],
                    )
                    nc.sync.dma_start(out=dst, in_=cc_out[c])

                tc.strict_bb_all_engine_barrier()

            # ---------------- Phase D: layer-2 aggregation + log_softmax -----
            if phases >= 3:
                with tc.tile_pool(name="ep2", bufs=4) as ep2pool:

                    def epi2(t, pt):
                        tmp = ep2pool.tile([P, FO], dt.float32, tag="tmp2")
                        nc.scalar.activation(
                            out=tmp[:],
                            in_=pt[:],
                            func=Act.Copy,
                            scale=dinvs[:, t : t + 1],
                        )
                        nc.vector.tensor_tensor(
                            out=tmp[:], in0=tmp[:], in1=b2r[:], op=Alu.add
                        )
                        mx = ep2pool.tile([P, 1], dt.float32, tag="mx")
                        nc.vector.reduce_max(
                            out=mx[:], in_=tmp[:], axis=mybir.AxisListType.X,
                            negate=True,
                        )
                        ex = ep2pool.tile([P, FO], dt.float32, tag="ex")
                        nc.scalar.activation(
                            out=ex[:], in_=tmp[:], func=Act.Exp, bias=mx[:, 0:1]
                        )
                        sm = ep2pool.tile([P, 1], dt.float32, tag="sm")
                        nc.vector.reduce_sum(
                            out=sm[:], in_=ex[:], axis=mybir.AxisListType.X
                        )
                        lg = ep2pool.tile([P, 1], dt.float32, tag="lg")
                        nc.scalar.activation(out=lg[:], in_=sm[:], func=Act.Ln)
                        nc.vector.tensor_scalar(
                            out=outst[:, t * FO : (t + 1) * FO],
                            in0=tmp[:],
                            scalar1=mx[:, 0:1],
                            scalar2=lg[:, 0:1],
                            op0=Alu.add,
                            op1=Alu.subtract,
                        )

                    aggregation(FO, F, FO, epi2)
                nc.sync.dma_start(out=out_e[:, :], in_=outst[:])
            else:
                nc.vector.memset(outst[:], 0.0)
                nc.sync.dma_start(out=out_e[:, :], in_=outst[:])

    _legalize_waits(nc)
    return nc


def kernel(x, edge_index, W1, b1, W2, b2, _trace=False, _trace_kwargs=None):
    in_maps, chunks, norder = _preprocess(x, edge_index, W1, b1, W2, b2)
    key = tuple(chunks)
    if key not in _CACHE:
        _CACHE[key] = _build(chunks)
    nc = _CACHE[key]

    res = run_bass_kernel_spmd(
        nc,
        in_maps,
        core_ids=list(range(NCORES)),
        trace=_trace,
        **(_trace_kwargs or {}),
    )
    out = np.empty((N, FO), dtype=np.float32)
    for c in range(NCORES):
        o = np.asarray(res.results[c]["out"], dtype=np.float32)
        o = o.reshape(P, NT, FO)  # [lane j, tile t, f]
        for t in range(NT):
            q0 = 1024 * t + P * c
            nodes = norder[q0 : q0 + P]
            m = nodes >= 0
            out[nodes[m]] = o[m, t]
    kernel._last_result = res
    return out


# revision 10
# speedup vs baseline: 1.7510x; 1.0073x over previous
"""GCN (2-layer, PyG GCNConv-style) on 8 Trainium2 NeuronCores.

v2 strategy — degree-sorted identity aggregation:
  - Nodes globally sorted by in-degree (random edges), padded to 50176
    positions; 128-position blocks dealt round-robin to cores (block b ->
    core b%8, tile b//8).  A dst tile therefore holds 128 near-equal-degree
    nodes, so per-tile "rounds" (one edge per dst lane per round) pad
    almost nothing:  rounds_t = 1 + max in-degree over the 8 sibling
    blocks, with a trailing self-loop round.
  - Aggregation is a per-round dma_gather of the 128 lanes' source rows
    (slot == lane) followed by an identity matmul accumulating into the
    tile's PSUM — no per-chunk selection-matrix builds at all.
  - The node table h_dram has one 256B row per position: bytes [0:128) =
    layer-1 features (64 bf16), bytes [128:160) = layer-2 features
    (16 bf16, written between layers).  Both layers share one int16 index
    table; the gather base is biased to row 32768 so signed indices cover
    all 50176 rows.  Gathers use raw InstDMAGatherAnt with elem_size 64
    (layer 1) / 16 (layer 2) elements and elem_step 128 (256B stride).
  - Pad slots (lanes whose degree < round count) and per-instruction
    flush chunks fetch the all-zero last pad row, keeping every gather
    instruction's trailing index non-negative (Q7 trims trailing
    negatives).
  - Phase A computes h1' = (dinv*x) @ W1 (x pre-scaled on host), batching
    7 tiles per PSUM bank; epilogues fold dinv_dst via activation scale;
    log_softmax on-chip; host un-permutes the output.
"""

import os
import sys

import numpy as np

for _p in ("/opt/trn_rl_repo", "/root/.axon_site/_ro/trn_rl_repo"):
    if os.path.isdir(_p) and _p not in sys.path:
        sys.path.insert(0, _p)

import ml_dtypes  # noqa: E402
import concourse.bass as bass  # noqa: E402
import concourse.mybir as mybir  # noqa: E402
import concourse.tile as tile  # noqa: E402
from concourse.bass_utils import run_bass_kernel_spmd  # noqa: E402
from concourse.masks import make_identity  # noqa: E402
from concourse import library_config  # noqa: E402
import concourse.bass_isa as bass_isa  # noqa: E402

# ---------------- static problem config (hardcoded per contract) -------------
N = 50000
E = 800000
F = 64          # F_IN == F_HID
FO = 16         # F_OUT
NCORES = 8
P = 128
NBLK = 392                # 128-position blocks
NPAD = NBLK * P           # 50176 positions
NT = NBLK // NCORES       # 49 tiles per core
ROWE = 128                # table row length in bf16 elements (256B stride)
BIAS = 32768              # gather base row (signed int16 indices)
PADPOS = NPAD - 1         # all-zero pad row
GI = 64                   # gather chunks per instruction
XCOLS = NPAD // 2         # 25088 columns per xpack half

BF16 = ml_dtypes.bfloat16

_CACHE = {}


def _pack_idx16(vals):
    """Slot-ordered int16 values [C*128] -> idx table [128, C*8].

    dma_gather reads index k from (partition k%16, col k//16), replicated
    across the 8 q7 cores (partition stripes of 16).
    """
    k = np.arange(vals.size)
    tbl = np.zeros((16, vals.size // 16), dtype=np.int16)
    tbl[k % 16, k // 16] = vals.astype(np.int16)
    return np.tile(tbl, (8, 1))


def _preprocess(x, edge_index, W1, b1, W2, b2):
    src = np.asarray(edge_index[0], dtype=np.int64)
    dst = np.asarray(edge_index[1], dtype=np.int64)

    rdeg = np.bincount(dst, minlength=N)
    dinv = (1.0 / np.sqrt(rdeg + 1.0)).astype(np.float32)

    order = np.argsort(-rdeg, kind="stable")          # node at each position
    norder = np.concatenate([order, np.full(NPAD - N, -1, dtype=np.int64)])
    pos = np.empty(N, dtype=np.int64)
    pos[order] = np.arange(N)

    posdinv = np.zeros(NPAD, dtype=np.float32)
    posdinv[pos] = dinv
    prdeg = np.zeros(NPAD, dtype=np.int64)
    prdeg[pos] = rdeg

    # per-position incoming-edge source lists (by position ids)
    pd = pos[dst]
    ps = pos[src]
    eorder = np.argsort(pd, kind="stable")
    ps_s = ps[eorder]
    starts = np.searchsorted(pd[eorder], np.arange(NPAD + 1))

    R = [1 + int(prdeg[1024 * t : 1024 * (t + 1)].max()) for t in range(NT)]

    # chunk stream structure (uniform across cores): (tile, round | -1=flush)
    chunks = []
    n = 0
    for t in range(NT):
        for r in range(R[t]):
            chunks.append((t, r))
            n += 1
            last = t == NT - 1 and r == R[t] - 1
            if n % GI == GI - 1 and not last:
                chunks.append((t, -1))
                n += 1
    Ctot = len(chunks)

    # per-core slot values (position ids; self round last)
    lanes = np.arange(P)
    idx16 = np.empty((NCORES, P, Ctot * 8), dtype=np.int16)
    for c in range(NCORES):
        vals = np.empty((Ctot, P), dtype=np.int64)
        for k, (t, r) in enumerate(chunks):
            if r < 0:
                vals[k] = PADPOS
                continue
            q = 1024 * t + P * c + lanes
            if r == R[t] - 1:
                vals[k] = q                      # self loop
            else:
                cnt = prdeg[q]
                v = np.full(P, PADPOS, dtype=np.int64)
                m = r < cnt
                v[m] = ps_s[starts[q[m]] + r]
                vals[k] = v
        idx16[c] = _pack_idx16((vals - BIAS).ravel())

    # xpack: x pre-scaled by dinv, transposed, two halves stacked on partitions
    xp = np.zeros((NPAD, F), dtype=np.float32)
    real = norder >= 0
    xp[real] = np.asarray(x, dtype=np.float32)[norder[real]] * posdinv[real, None]
    xpack = xp.reshape(2, XCOLS, F).transpose(0, 2, 1).reshape(P, XCOLS).astype(BF16)

    dinvs = np.zeros((NCORES, P, NT), dtype=np.float32)
    for c in range(NCORES):
        for t in range(NT):
            dinvs[c, :, t] = posdinv[1024 * t + P * c + lanes]

    common = {
        "xpack": xpack,
        "w1": np.concatenate([np.asarray(W1, np.float32)] * 2, axis=0).astype(BF16),
        "w2": np.asarray(W2, dtype=np.float32).astype(BF16),
        "b1r": np.broadcast_to(np.asarray(b1, np.float32), (P, F)).copy(),
        "b2r": np.broadcast_to(np.asarray(b2, np.float32), (P, FO)).copy(),
    }
    in_maps = []
    for c in range(NCORES):
        m = dict(common)
        m["dinvs"] = dinvs[c]
        m["idx16"] = idx16[c]
        in_maps.append(m)
    return in_maps, chunks, norder


_WAIT_LIMIT = int(os.environ.get("GCN_WAIT_LIMIT", "1"))


def _legalize_waits(nc, limit=None):
    """Split >limit semaphore waits into standalone NOPs on the same engine.

    Walrus codegen rejects instructions whose sync_info carries more wait
    conditions than the ISA sync fields hold ("Too many sync wait commands").
    A chain of no-ops each carrying <=limit waits is semantically identical
    (waits are AND conditions and the engine queue is in-order).
    """
    if limit is None:
        limit = _WAIT_LIMIT
    import bass_rust as _br

    uid = 0
    for fn in nc.m.functions:
        for bb in fn.blocks:
            out = []
            changed = False
            for ins in bb.instructions:
                si = ins.sync_info
                if si is not None and len(si.on_wait) > limit:
                    waits = list(si.on_wait)
                    excess, keep = waits[:-limit], waits[-limit:]
                    for i in range(0, len(excess), limit):
                        nop = mybir.InstNoOp(name=f"waitsplit_{uid}", ins=[], outs=[])
                        uid += 1
                        nop.engine = ins.engine
                        nop.sync_info = _br.SyncInfo(
                            on_wait=excess[i : i + limit], on_update=[]
                        )
                        out.append(nop)
                    ins.sync_info = _br.SyncInfo(
                        on_wait=keep, on_update=list(si.on_update)
                    )
                    changed = True
                out.append(ins)
            if changed:
                bb.instructions = out


def _dma_gather_raw(nc, out_ap, in_ap, idxs_ap, num_idxs, elem_size, elem_step):
    """dma_gather with elem_size not a multiple of 256B (bass.py over-asserts
    the transpose-path alignment).  Mirrors the tail of BassGpSimd.dma_gather
    for the DRAM-source, transpose=False case: per-index descriptors read
    elem_size elements from base + idx*elem_step (stride must be 256B-aligned,
    elem_size is free)."""
    eng = nc.gpsimd
    stride_bytes = elem_step * mybir.dt.size(in_ap.dtype)
    assert stride_bytes % 256 == 0
    _in_ap = eng.lower_ap_dma(in_ap, for_custom_bir_dma=True)
    _idxs_ap = eng.lower_ap(idxs_ap)
    _out_ap = eng.lower_ap(out_ap)
    return eng.add_instruction(
        mybir.InstDMAGatherAnt(
            name=nc.get_next_instruction_name(),
            ins=[*_in_ap, _idxs_ap, eng.lower_val_access(eng.to_reg(num_idxs))],
            outs=[_out_ap],
            transpose=False,
            num_idxs=num_idxs,
            elem_size=elem_size,
            stride_bytes_256=stride_bytes // 256,
            gen_mode=0,
            single_packet=False,
            queue_num=0,
            sbuf_tokens_per_rank=0,
            sbuf_free_dim_per_rank=0,
            sbuf_free_dim_pad_per_rank=0,
            sbuf_byte_offset=0,
        )
    )


def _build(chunks):
    dt = mybir.dt
    Alu = mybir.AluOpType
    Act = mybir.ActivationFunctionType

    Ctot = len(chunks)
    # gather instruction groups [c0, c1)
    groups = [(g, min(g + GI, Ctot)) for g in range(0, Ctot, GI)]
    # per-tile chunk index lists
    tchunks = [[] for _ in range(NT)]
    for k, (t, _r) in enumerate(chunks):
        tchunks[t].append(k)

    nc = bass.Bass(num_devices=NCORES)

    xpack_e = nc.dram_tensor("xpack", [P, XCOLS], dt.bfloat16, kind="ExternalInput")
    w1_e = nc.dram_tensor("w1", [2 * F, F], dt.bfloat16, kind="ExternalInput")
    w2_e = nc.dram_tensor("w2", [F, FO], dt.bfloat16, kind="ExternalInput")
    b1_e = nc.dram_tensor("b1r", [P, F], dt.float32, kind="ExternalInput")
    b2_e = nc.dram_tensor("b2r", [P, FO], dt.float32, kind="ExternalInput")
    dinvs_e = nc.dram_tensor("dinvs", [P, NT], dt.float32, kind="ExternalInput")
    idx16_e = nc.dram_tensor("idx16", [P, Ctot * 8], dt.int16, kind="ExternalInput")
    out_e = nc.dram_tensor("out", [P, NT * FO], dt.float32, kind="ExternalOutput")
    phases = int(os.environ.get("GCN_PHASES", "3"))

    h_dram = nc.dram_tensor("h_dram", [NPAD, ROWE], dt.bfloat16)
    cc_in = nc.dram_tensor("cc_in", [P, NT * FO], dt.bfloat16)
    cc_out = nc.dram_tensor(
        "cc_out", [NCORES, P, NT * FO], dt.bfloat16, addr_space="Shared"
    )

    with tile.TileContext(nc) as tc:
        with tc.tile_pool(name="const", bufs=1) as cp:
            w1 = cp.tile([2 * F, F], dt.bfloat16, tag="w1")
            nc.sync.dma_start(out=w1[:], in_=w1_e[:, :])
            w2 = cp.tile([F, FO], dt.bfloat16, tag="w2")
            nc.sync.dma_start(out=w2[:], in_=w2_e[:, :])
            b1r = cp.tile([P, F], dt.float32, tag="b1r")
            nc.sync.dma_start(out=b1r[:], in_=b1_e[:, :])
            b2r = cp.tile([P, FO], dt.float32, tag="b2r")
            nc.sync.dma_start(out=b2r[:], in_=b2_e[:, :])
            dinvs = cp.tile([P, NT], dt.float32, tag="dinvs")
            nc.sync.dma_start(out=dinvs[:], in_=dinvs_e[:, :])
            idx16 = cp.tile([P, Ctot * 8], dt.int16, tag="idx16")
            ident = cp.tile([P, P], dt.bfloat16, tag="ident")
            make_identity(nc, ident[:])
            out1 = cp.tile([P, NT * F], dt.bfloat16, tag="out1")
            h2st = cp.tile([P, NT * FO], dt.bfloat16, tag="h2st")
            outst = cp.tile([P, NT * FO], dt.float32, tag="outst")

            tc.strict_bb_all_engine_barrier()
            # dma_gather lives in the Q7 "mlp" extended-instruction library.
            # bass's pseudo reload ships with an empty instr payload, which
            # walrus rejects ("ISA wrong length") — fill the 64B struct.
            _li = nc.gpsimd.load_library(library_config.mlp)
            _instr, _fx = bass_isa.isa_struct(
                nc.isa,
                nc.isa.Opcode.NEURON_ISA_TPB_OPCODE_PSEUDO_INST,
                {"pseudo_opcode": 2, "lib_index": library_config.mlp.index},
                struct_name="NEURON_ISA_TPB_PSEUDO_LIBRARY_RELOAD_INDEX_STRUCT",
            )
            _li.ins.instr = _instr

            # ---------------- Phase A: h1' = (dinv*x) @ W1 -> table ----------
            XB = 1792  # 14 tiles per (block, half); 14 blocks
            TB = 7     # tiles batched per PSUM bank
            with (
                tc.tile_pool(name="xa", bufs=3) as xpool,
                tc.tile_pool(name="ha", bufs=3) as hpool,
                tc.tile_pool(name="pha", bufs=3, space="PSUM") as phpool,
            ):
                for blk in range(0, XCOLS, XB):
                    xb = xpool.tile([P, XB], dt.bfloat16, tag="xb")
                    nc.sync.dma_start(out=xb[:], in_=xpack_e[:, blk : blk + XB])
                    for a in (0, 1):
                        for u0 in range(0, XB // P, TB):
                            ph = phpool.tile([P, TB * F], dt.float32, tag="ph")
                            for m in range(TB):
                                mm = u0 + m
                                nc.tensor.matmul(
                                    out=ph[:, m * F : (m + 1) * F],
                                    lhsT=xb[a * F : (a + 1) * F,
                                            mm * P : (mm + 1) * P],
                                    rhs=w1[a * F : (a + 1) * F, :],
                                    start=True,
                                    stop=True,
                                )
                            hst = hpool.tile([P, TB * F], dt.bfloat16, tag="hst")
                            nc.scalar.activation(out=hst[:], in_=ph[:], func=Act.Copy)
                            g0 = a * (XCOLS // P) + blk // P + u0
                            dst = bass.AP(
                                h_dram,
                                g0 * P * ROWE,
                                [[ROWE, P], [P * ROWE, TB], [1, F]],
                            )
                            nc.sync.dma_start(out=dst, in_=hst[:])

            # loaded late so Phase A's first x block isn't queued behind it
            nc.sync.dma_start(out=idx16[:], in_=idx16_e[:, :])
            tc.strict_bb_all_engine_barrier()

            # ---------------- aggregation machinery --------------------------
            def aggregation(elem, col0, fw, epilogue):
                """Identity-matmul rounds over the shared chunk stream."""
                src = bass.AP(
                    h_dram,
                    BIAS * ROWE + col0,
                    [[ROWE, NPAD - BIAS], [1, elem]],
                )
                with (
                    tc.tile_pool(name="gb", bufs=3) as gpool,
                    tc.tile_pool(name="pagg", bufs=4, space="PSUM") as ppool,
                ):
                    gbufs = [None] * len(groups)

                    def issue(g):
                        c0, c1 = groups[g]
                        w = c1 - c0
                        gb = gpool.tile([P, GI * elem], dt.bfloat16, tag="gb")
                        _dma_gather_raw(
                            nc,
                            out_ap=gb[:, : w * elem].rearrange(
                                "p (s e) -> p s e", e=elem
                            ),
                            in_ap=src,
                            idxs_ap=idx16[:, c0 * 8 : c1 * 8],
                            num_idxs=w * P,
                            elem_size=elem,
                            elem_step=ROWE,
                        )
                        gbufs[g] = gb

                    issued = 0
                    for t in range(NT):
                        ks = tchunks[t]
                        while issued * GI < ks[-1] + 1:
                            issue(issued)
                            issued += 1
                        pt = ppool.tile([P, fw], dt.float32, tag="pt")
                        for i, k in enumerate(ks):
                            g, kl = k // GI, k % GI
                            nc.tensor.matmul(
                                out=pt[:],
                                lhsT=ident[:],
                                rhs=gbufs[g][:, kl * elem : kl * elem + fw],
                                start=(i == 0),
                                stop=(i == len(ks) - 1),
                            )
                        epilogue(t, pt)

            # ---------------- Phase B: layer-1 aggregation -------------------
            if phases >= 1:
                with tc.tile_pool(name="ep1", bufs=4) as ep1pool:

                    def epi1(t, pt):
                        tmp = ep1pool.tile([P, F], dt.float32, tag="tmp")
                        nc.scalar.activation(
                            out=tmp[:],
                            in_=pt[:],
                            func=Act.Copy,
                            scale=dinvs[:, t : t + 1],
                        )
                        nc.vector.tensor_tensor(
                            out=tmp[:], in0=tmp[:], in1=b1r[:], op=Alu.add
                        )
                        nc.scalar.activation(
                            out=out1[:, t * F : (t + 1) * F], in_=tmp[:],
                            func=Act.Relu,
                        )

                    aggregation(F, 0, F, epi1)

            # ---------------- Phase C: h2' = dinv * (out1 @ W2) -> table -----
            if phases >= 2:
                with (
                    tc.tile_pool(name="ptrp", bufs=2, space="PSUM") as ptrpool,
                    tc.tile_pool(name="ph2p", bufs=2, space="PSUM") as ph2pool,
                    tc.tile_pool(name="o1tp", bufs=2) as o1tpool,
                ):
                    for t in range(NT):
                        ptr_ = ptrpool.tile([P, P], dt.bfloat16, tag="ptr")
                        nc.tensor.transpose(
                            out=ptr_[:F, :],
                            in_=out1[:, t * F : (t + 1) * F],
                            identity=ident[:],
                        )
                        o1T = o1tpool.tile([F, P], dt.bfloat16, tag="o1T")
                        nc.vector.tensor_copy(out=o1T[:], in_=ptr_[:F, :])
                        ph2 = ph2pool.tile([P, FO], dt.float32, tag="ph2")
                        nc.tensor.matmul(
                            out=ph2[:], lhsT=o1T[:], rhs=w2[:, :],
                            start=True, stop=True,
                        )
                        nc.scalar.activation(
                            out=h2st[:, t * FO : (t + 1) * FO],
                            in_=ph2[:],
                            func=Act.Copy,
                            scale=dinvs[:, t : t + 1],
                        )
                    nc.sync.dma_start(out=cc_in[:, :], in_=h2st[:])

            if phases >= 2 and not int(os.environ.get("GCN_NO_CC", "0")):
                SL = P * NT * FO
                nc.gpsimd.collective_compute(
                    "AllGather",
                    mybir.AluOpType.bypass,
                    replica_groups=[list(range(NCORES))],
                    ins=[cc_in.ap()],
                    outs=[bass.AP(cc_out, 0, [[SL, NCORES], [1, SL]])],
                )

            if phases >= 2 and not int(os.environ.get("GCN_NO_SCATTER", "0")):
                # one DMA scatters all cores' h2 into the table rows:
                # position q = 1024t + 128c + j, row elements [64:80)
                dst = bass.AP(
                    h_dram,
                    F,
                    [[P * ROWE, NCORES], [ROWE, P], [1024 * ROWE, NT], [1, FO]],
                )
                nc.sync.dma_start(out=dst, in_=cc_out[:, :, :])
                tc.strict_bb_all_engine_barrier()

            # ---------------- Phase D: layer-2 aggregation + log_softmax -----
            if phases >= 3:
                with tc.tile_pool(name="ep2", bufs=4) as ep2pool:

                    def epi2(t, pt):
                        tmp = ep2pool.tile([P, FO], dt.float32, tag="tmp2")
                        nc.scalar.activation(
                            out=tmp[:],
                            in_=pt[:],
                            func=Act.Copy,
                            scale=dinvs[:, t : t + 1],
                        )
                        nc.vector.tensor_tensor(
                            out=tmp[:], in0=tmp[:], in1=b2r[:], op=Alu.add
                        )
                        mx = ep2pool.tile([P, 1], dt.float32, tag="mx")
                        nc.vector.reduce_max(
                            out=mx[:], in_=tmp[:], axis=mybir.AxisListType.X,
                            negate=True,
                        )
                        ex = ep2pool.tile([P, FO], dt.float32, tag="ex")
                        nc.scalar.activation(
                            out=ex[:], in_=tmp[:], func=Act.Exp, bias=mx[:, 0:1]
                        )
                        sm = ep2pool.tile([P, 1], dt.float32, tag="sm")
                        nc.vector.reduce_sum(
                            out=sm[:], in_=ex[:], axis=mybir.AxisListType.X
                        )
                        lg = ep2pool.tile([P, 1], dt.float32, tag="lg")
                        nc.scalar.activation(out=lg[:], in_=sm[:], func=Act.Ln)
                        nc.vector.tensor_scalar(
                            out=outst[:, t * FO : (t + 1) * FO],
                            in0=tmp[:],
                            scalar1=mx[:, 0:1],
                            scalar2=lg[:, 0:1],
                            op0=Alu.add,
                            op1=Alu.subtract,
                        )

                    aggregation(FO, F, FO, epi2)
                nc.sync.dma_start(out=out_e[:, :], in_=outst[:])
            else:
                nc.vector.memset(outst[:], 0.0)
                nc.sync.dma_start(out=out_e[:, :], in_=outst[:])

    _legalize_waits(nc)
    return nc


def kernel(x, edge_index, W1, b1, W2, b2, _trace=False, _trace_kwargs=None):
    in_maps, chunks, norder = _preprocess(x, edge_index, W1, b1, W2, b2)
    key = tuple(chunks)
    if key not in _CACHE:
        _CACHE[key] = _build(chunks)
    nc = _CACHE[key]

    res = run_bass_kernel_spmd(
        nc,
        in_maps,
        core_ids=list(range(NCORES)),
        trace=_trace,
        **(_trace_kwargs or {}),
    )
    out = np.empty((N, FO), dtype=np.float32)
    for c in range(NCORES):
        o = np.asarray(res.results[c]["out"], dtype=np.float32)
        o = o.reshape(P, NT, FO)  # [lane j, tile t, f]
        for t in range(NT):
            q0 = 1024 * t + P * c
            nodes = norder[q0 : q0 + P]
            m = nodes >= 0
            out[nodes[m]] = o[m, t]
    kernel._last_result = res
    return out


# revision 17
# speedup vs baseline: 1.8707x; 1.0684x over previous
"""GCN (2-layer, PyG GCNConv-style) on 8 Trainium2 NeuronCores.

v2 strategy — degree-sorted identity aggregation:
  - Nodes globally sorted by in-degree (random edges), padded to 50176
    positions; 128-position blocks dealt round-robin to cores (block b ->
    core b%8, tile b//8).  A dst tile therefore holds 128 near-equal-degree
    nodes, so per-tile "rounds" (one edge per dst lane per round) pad
    almost nothing:  rounds_t = 1 + max in-degree over the 8 sibling
    blocks, with a trailing self-loop round.
  - Aggregation is a per-round dma_gather of the 128 lanes' source rows
    (slot == lane) followed by an identity matmul accumulating into the
    tile's PSUM — no per-chunk selection-matrix builds at all.
  - The node table h_dram has one 256B row per position: bytes [0:128) =
    layer-1 features (64 bf16), bytes [128:160) = layer-2 features
    (16 bf16, written between layers).  Both layers share one int16 index
    table; the gather base is biased to row 32768 so signed indices cover
    all 50176 rows.  Gathers use raw InstDMAGatherAnt with elem_size 64
    (layer 1) / 16 (layer 2) elements and elem_step 128 (256B stride).
  - Pad slots (lanes whose degree < round count) and per-instruction
    flush chunks fetch the all-zero last pad row, keeping every gather
    instruction's trailing index non-negative (Q7 trims trailing
    negatives).
  - Phase A computes h1' = (dinv*x) @ W1 (x pre-scaled on host), batching
    7 tiles per PSUM bank; epilogues fold dinv_dst via activation scale;
    log_softmax on-chip; host un-permutes the output.
"""

import os
import sys

import numpy as np

for _p in ("/opt/trn_rl_repo", "/root/.axon_site/_ro/trn_rl_repo"):
    if os.path.isdir(_p) and _p not in sys.path:
        sys.path.insert(0, _p)

import ml_dtypes  # noqa: E402
import concourse.bass as bass  # noqa: E402
import concourse.mybir as mybir  # noqa: E402
import concourse.tile as tile  # noqa: E402
from concourse.bass_utils import run_bass_kernel_spmd  # noqa: E402
from concourse.masks import make_identity  # noqa: E402
from concourse import library_config  # noqa: E402
import concourse.bass_isa as bass_isa  # noqa: E402

# ---------------- static problem config (hardcoded per contract) -------------
N = 50000
E = 800000
F = 64          # F_IN == F_HID
FO = 16         # F_OUT
NCORES = 8
P = 128
NBLK = 392                # 128-position blocks
NPAD = NBLK * P           # 50176 positions
NT = NBLK // NCORES       # 49 tiles per core
ROWE = 128                # table row length in bf16 elements (256B stride)
BIAS = 32768              # gather base row (signed int16 indices)
PADPOS = NPAD - 1         # all-zero pad row
GI = 64                   # gather chunks per instruction
XCOLS = NPAD // 2         # 25088 columns per xpack half

BF16 = ml_dtypes.bfloat16

_CACHE = {}


def _pack_idx16(vals):
    """Slot-ordered int16 values [C*128] -> idx table [128, C*8].

    dma_gather reads index k from (partition k%16, col k//16), replicated
    across the 8 q7 cores (partition stripes of 16).
    """
    k = np.arange(vals.size)
    tbl = np.zeros((16, vals.size // 16), dtype=np.int16)
    tbl[k % 16, k // 16] = vals.astype(np.int16)
    return np.tile(tbl, (8, 1))


def _preprocess(x, edge_index, W1, b1, W2, b2):
    src = np.asarray(edge_index[0], dtype=np.int64)
    dst = np.asarray(edge_index[1], dtype=np.int64)

    rdeg = np.bincount(dst, minlength=N)
    dinv = (1.0 / np.sqrt(rdeg + 1.0)).astype(np.float32)

    order = np.argsort(-rdeg, kind="stable")          # node at each position
    norder = np.concatenate([order, np.full(NPAD - N, -1, dtype=np.int64)])
    pos = np.empty(N, dtype=np.int64)
    pos[order] = np.arange(N)

    posdinv = np.zeros(NPAD, dtype=np.float32)
    posdinv[pos] = dinv
    prdeg = np.zeros(NPAD, dtype=np.int64)
    prdeg[pos] = rdeg

    # per-position incoming-edge source lists (by position ids)
    pd = pos[dst]
    ps = pos[src]
    eorder = np.argsort(pd, kind="stable")
    ps_s = ps[eorder]
    starts = np.searchsorted(pd[eorder], np.arange(NPAD + 1))

    R = [1 + int(prdeg[1024 * t : 1024 * (t + 1)].max()) for t in range(NT)]

    # chunk stream structure (uniform across cores): (tile, round | -1=flush)
    chunks = []
    n = 0
    for t in range(NT):
        for r in range(R[t]):
            chunks.append((t, r))
            n += 1
            last = t == NT - 1 and r == R[t] - 1
            if n % GI == GI - 1 and not last:
                chunks.append((t, -1))
                n += 1
    Ctot = len(chunks)

    # per-core slot values (position ids; self round last)
    lanes = np.arange(P)
    idx16 = np.empty((NCORES, P, Ctot * 8), dtype=np.int16)
    for c in range(NCORES):
        vals = np.empty((Ctot, P), dtype=np.int64)
        for k, (t, r) in enumerate(chunks):
            if r < 0:
                vals[k] = PADPOS
                continue
            q = 1024 * t + P * c + lanes
            if r == R[t] - 1:
                vals[k] = q                      # self loop
            else:
                cnt = prdeg[q]
                v = np.full(P, PADPOS, dtype=np.int64)
                m = r < cnt
                v[m] = ps_s[starts[q[m]] + r]
                vals[k] = v
        idx16[c] = _pack_idx16((vals - BIAS).ravel())

    # xpack: x pre-scaled by dinv, transposed, two halves stacked on partitions
    xp = np.zeros((NPAD, F), dtype=np.float32)
    real = norder >= 0
    xp[real] = np.asarray(x, dtype=np.float32)[norder[real]] * posdinv[real, None]
    xpack = xp.reshape(2, XCOLS, F).transpose(0, 2, 1).reshape(P, XCOLS).astype(BF16)

    dinvs = np.zeros((NCORES, P, NT), dtype=np.float32)
    for c in range(NCORES):
        for t in range(NT):
            dinvs[c, :, t] = posdinv[1024 * t + P * c + lanes]

    common = {
        "xpack": xpack,
        "w1": np.concatenate([np.asarray(W1, np.float32)] * 2, axis=0).astype(BF16),
        "w2": np.asarray(W2, dtype=np.float32).astype(BF16),
        "b1r": np.broadcast_to(np.asarray(b1, np.float32), (P, F)).copy(),
        "b2r": np.broadcast_to(np.asarray(b2, np.float32), (P, FO)).copy(),
    }
    in_maps = []
    for c in range(NCORES):
        m = dict(common)
        m["dinvs"] = dinvs[c]
        m["idx16"] = idx16[c]
        in_maps.append(m)
    return in_maps, chunks, norder


_WAIT_LIMIT = int(os.environ.get("GCN_WAIT_LIMIT", "1"))


def _legalize_waits(nc, limit=None):
    """Split >limit semaphore waits into standalone NOPs on the same engine.

    Walrus codegen rejects instructions whose sync_info carries more wait
    conditions than the ISA sync fields hold ("Too many sync wait commands").
    A chain of no-ops each carrying <=limit waits is semantically identical
    (waits are AND conditions and the engine queue is in-order).
    """
    if limit is None:
        limit = _WAIT_LIMIT
    import bass_rust as _br

    uid = 0
    for fn in nc.m.functions:
        for bb in fn.blocks:
            out = []
            changed = False
            for ins in bb.instructions:
                si = ins.sync_info
                if si is not None and len(si.on_wait) > limit:
                    waits = list(si.on_wait)
                    excess, keep = waits[:-limit], waits[-limit:]
                    for i in range(0, len(excess), limit):
                        nop = mybir.InstNoOp(name=f"waitsplit_{uid}", ins=[], outs=[])
                        uid += 1
                        nop.engine = ins.engine
                        nop.sync_info = _br.SyncInfo(
                            on_wait=excess[i : i + limit], on_update=[]
                        )
                        out.append(nop)
                    ins.sync_info = _br.SyncInfo(
                        on_wait=keep, on_update=list(si.on_update)
                    )
                    changed = True
                out.append(ins)
            if changed:
                bb.instructions = out


def _dma_gather_raw(nc, out_ap, in_ap, idxs_ap, num_idxs, elem_size, elem_step):
    """dma_gather with elem_size not a multiple of 256B (bass.py over-asserts
    the transpose-path alignment).  Mirrors the tail of BassGpSimd.dma_gather
    for the DRAM-source, transpose=False case: per-index descriptors read
    elem_size elements from base + idx*elem_step (stride must be 256B-aligned,
    elem_size is free)."""
    eng = nc.gpsimd
    stride_bytes = elem_step * mybir.dt.size(in_ap.dtype)
    assert stride_bytes % 256 == 0
    _in_ap = eng.lower_ap_dma(in_ap, for_custom_bir_dma=True)
    _idxs_ap = eng.lower_ap(idxs_ap)
    _out_ap = eng.lower_ap(out_ap)
    return eng.add_instruction(
        mybir.InstDMAGatherAnt(
            name=nc.get_next_instruction_name(),
            ins=[*_in_ap, _idxs_ap, eng.lower_val_access(eng.to_reg(num_idxs))],
            outs=[_out_ap],
            transpose=False,
            num_idxs=num_idxs,
            elem_size=elem_size,
            stride_bytes_256=stride_bytes // 256,
            gen_mode=0,
            single_packet=False,
            queue_num=0,
            sbuf_tokens_per_rank=0,
            sbuf_free_dim_per_rank=0,
            sbuf_free_dim_pad_per_rank=0,
            sbuf_byte_offset=0,
        )
    )


def _build(chunks):
    dt = mybir.dt
    Alu = mybir.AluOpType
    Act = mybir.ActivationFunctionType

    Ctot = len(chunks)
    # gather instruction groups [c0, c1)
    groups = [(g, min(g + GI, Ctot)) for g in range(0, Ctot, GI)]
    # per-tile chunk index lists
    tchunks = [[] for _ in range(NT)]
    for k, (t, _r) in enumerate(chunks):
        tchunks[t].append(k)

    nc = bass.Bass(num_devices=NCORES)

    xpack_e = nc.dram_tensor("xpack", [P, XCOLS], dt.bfloat16, kind="ExternalInput")
    w1_e = nc.dram_tensor("w1", [2 * F, F], dt.bfloat16, kind="ExternalInput")
    w2_e = nc.dram_tensor("w2", [F, FO], dt.bfloat16, kind="ExternalInput")
    b1_e = nc.dram_tensor("b1r", [P, F], dt.float32, kind="ExternalInput")
    b2_e = nc.dram_tensor("b2r", [P, FO], dt.float32, kind="ExternalInput")
    dinvs_e = nc.dram_tensor("dinvs", [P, NT], dt.float32, kind="ExternalInput")
    idx16_e = nc.dram_tensor("idx16", [P, Ctot * 8], dt.int16, kind="ExternalInput")
    out_e = nc.dram_tensor("out", [P, NT * FO], dt.float32, kind="ExternalOutput")
    phases = int(os.environ.get("GCN_PHASES", "3"))

    # layer-1 table: fp8e3m4 rows, 256B stride, payload [0:64); layer-2
    # table: bf16 rows, 256B stride, payload [0:16).  Same row ids.
    h_dram = nc.dram_tensor("h_dram", [NPAD, 256], dt.float8e3)
    h2_dram = nc.dram_tensor("h2_dram", [NPAD, ROWE], dt.bfloat16)
    cc_in = nc.dram_tensor("cc_in", [P, NT * FO], dt.bfloat16)
    cc_out = nc.dram_tensor(
        "cc_out", [NCORES, P, NT * FO], dt.bfloat16, addr_space="Shared"
    )

    with tile.TileContext(nc) as tc:
        with tc.tile_pool(name="const", bufs=1) as cp:
            w1 = cp.tile([2 * F, F], dt.bfloat16, tag="w1")
            nc.sync.dma_start(out=w1[:], in_=w1_e[:, :])
            w2 = cp.tile([F, FO], dt.bfloat16, tag="w2")
            nc.sync.dma_start(out=w2[:], in_=w2_e[:, :])
            b1r = cp.tile([P, F], dt.float32, tag="b1r")
            nc.sync.dma_start(out=b1r[:], in_=b1_e[:, :])
            b2r = cp.tile([P, FO], dt.float32, tag="b2r")
            nc.sync.dma_start(out=b2r[:], in_=b2_e[:, :])
            dinvs = cp.tile([P, NT], dt.float32, tag="dinvs")
            nc.sync.dma_start(out=dinvs[:], in_=dinvs_e[:, :])
            idx16 = cp.tile([P, Ctot * 8], dt.int16, tag="idx16")
            ident = cp.tile([P, P], dt.bfloat16, tag="ident")
            make_identity(nc, ident[:])
            ident8 = cp.tile([P, P], dt.float8e3, tag="ident8")
            make_identity(nc, ident8[:])
            out1 = cp.tile([P, NT * F], dt.bfloat16, tag="out1")
            h2st = cp.tile([P, NT * FO], dt.bfloat16, tag="h2st")
            outst = cp.tile([P, NT * FO], dt.float32, tag="outst")

            tc.strict_bb_all_engine_barrier()
            # dma_gather lives in the Q7 "mlp" extended-instruction library.
            # bass's pseudo reload ships with an empty instr payload, which
            # walrus rejects ("ISA wrong length") — fill the 64B struct.
            _li = nc.gpsimd.load_library(library_config.mlp)
            _instr, _fx = bass_isa.isa_struct(
                nc.isa,
                nc.isa.Opcode.NEURON_ISA_TPB_OPCODE_PSEUDO_INST,
                {"pseudo_opcode": 2, "lib_index": library_config.mlp.index},
                struct_name="NEURON_ISA_TPB_PSEUDO_LIBRARY_RELOAD_INDEX_STRUCT",
            )
            _li.ins.instr = _instr

            # ---------------- Phase A: h1' = (dinv*x) @ W1 -> table ----------
            XB = 1792  # 14 tiles per (block, half); 14 blocks
            TB = 7     # tiles batched per PSUM bank
            with (
                tc.tile_pool(name="xa", bufs=3) as xpool,
                tc.tile_pool(name="ha", bufs=3) as hpool,
                tc.tile_pool(name="pha", bufs=3, space="PSUM") as phpool,
            ):
                for blk in range(0, XCOLS, XB):
                    xb = xpool.tile([P, XB], dt.bfloat16, tag="xb")
                    nc.sync.dma_start(out=xb[:], in_=xpack_e[:, blk : blk + XB])
                    for a in (0, 1):
                        for u0 in range(0, XB // P, TB):
                            ph = phpool.tile([P, TB * F], dt.float32, tag="ph")
                            for m in range(TB):
                                mm = u0 + m
                                nc.tensor.matmul(
                                    out=ph[:, m * F : (m + 1) * F],
                                    lhsT=xb[a * F : (a + 1) * F,
                                            mm * P : (mm + 1) * P],
                                    rhs=w1[a * F : (a + 1) * F, :],
                                    start=True,
                                    stop=True,
                                )
                            hst = hpool.tile([P, TB * F], dt.float8e3, tag="hst")
                            nc.scalar.activation(out=hst[:], in_=ph[:], func=Act.Copy)
                            g0 = a * (XCOLS // P) + blk // P + u0
                            dst = bass.AP(
                                h_dram,
                                g0 * P * 256,
                                [[256, P], [P * 256, TB], [1, F]],
                            )
                            nc.sync.dma_start(out=dst, in_=hst[:])

            # loaded late so Phase A's first x block isn't queued behind it
            nc.sync.dma_start(out=idx16[:], in_=idx16_e[:, :])
            tc.strict_bb_all_engine_barrier()

            # ---------------- aggregation machinery --------------------------
            def aggregation(table, rowlen, gdt, lhsT, elem, fw, epilogue):
                """Identity-matmul rounds over the shared chunk stream."""
                src = bass.AP(
                    table,
                    BIAS * rowlen,
                    [[rowlen, NPAD - BIAS], [1, elem]],
                )
                with (
                    tc.tile_pool(name="gb", bufs=3) as gpool,
                    tc.tile_pool(name="pagg", bufs=4, space="PSUM") as ppool,
                ):
                    gbufs = [None] * len(groups)

                    def issue(g):
                        c0, c1 = groups[g]
                        w = c1 - c0
                        gb = gpool.tile([P, GI * elem], gdt, tag="gb")
                        _dma_gather_raw(
                            nc,
                            out_ap=gb[:, : w * elem].rearrange(
                                "p (s e) -> p s e", e=elem
                            ),
                            in_ap=src,
                            idxs_ap=idx16[:, c0 * 8 : c1 * 8],
                            num_idxs=w * P,
                            elem_size=elem,
                            elem_step=rowlen,
                        )
                        gbufs[g] = gb

                    issued = 0
                    for t in range(NT):
                        ks = tchunks[t]
                        while issued * GI < ks[-1] + 1:
                            issue(issued)
                            issued += 1
                        pt = ppool.tile([P, fw], dt.float32, tag="pt")
                        for i, k in enumerate(ks):
                            g, kl = k // GI, k % GI
                            nc.tensor.matmul(
                                out=pt[:],
                                lhsT=lhsT[:],
                                rhs=gbufs[g][:, kl * elem : kl * elem + fw],
                                start=(i == 0),
                                stop=(i == len(ks) - 1),
                            )
                        epilogue(t, pt)

            # ---------------- Phase B: layer-1 aggregation -------------------
            if phases >= 1:
                with tc.tile_pool(name="ep1", bufs=4) as ep1pool:

                    def epi1(t, pt):
                        tmp = ep1pool.tile([P, F], dt.float32, tag="tmp")
                        nc.scalar.activation(
                            out=tmp[:],
                            in_=pt[:],
                            func=Act.Copy,
                            scale=dinvs[:, t : t + 1],
                        )
                        nc.vector.tensor_tensor(
                            out=tmp[:], in0=tmp[:], in1=b1r[:], op=Alu.add
                        )
                        nc.scalar.activation(
                            out=out1[:, t * F : (t + 1) * F], in_=tmp[:],
                            func=Act.Relu,
                        )

                    aggregation(h_dram, 256, dt.float8e3, ident8, F, F, epi1)

            # ---------------- Phase C: h2' = dinv * (out1 @ W2) -> table -----
            if phases >= 2:
                with (
                    tc.tile_pool(name="ptrp", bufs=2, space="PSUM") as ptrpool,
                    tc.tile_pool(name="ph2p", bufs=2, space="PSUM") as ph2pool,
                    tc.tile_pool(name="o1tp", bufs=2) as o1tpool,
                ):
                    for t in range(NT):
                        ptr_ = ptrpool.tile([P, P], dt.bfloat16, tag="ptr")
                        nc.tensor.transpose(
                            out=ptr_[:F, :],
                            in_=out1[:, t * F : (t + 1) * F],
                            identity=ident[:],
                        )
                        o1T = o1tpool.tile([F, P], dt.bfloat16, tag="o1T")
                        nc.vector.tensor_copy(out=o1T[:], in_=ptr_[:F, :])
                        ph2 = ph2pool.tile([P, FO], dt.float32, tag="ph2")
                        nc.tensor.matmul(
                            out=ph2[:], lhsT=o1T[:], rhs=w2[:, :],
                            start=True, stop=True,
                        )
                        nc.scalar.activation(
                            out=h2st[:, t * FO : (t + 1) * FO],
                            in_=ph2[:],
                            func=Act.Copy,
                            scale=dinvs[:, t : t + 1],
                        )
                    nc.sync.dma_start(out=cc_in[:, :], in_=h2st[:])

            if phases >= 2 and not int(os.environ.get("GCN_NO_CC", "0")):
                SL = P * NT * FO
                nc.gpsimd.collective_compute(
                    "AllGather",
                    mybir.AluOpType.bypass,
                    replica_groups=[list(range(NCORES))],
                    ins=[cc_in.ap()],
                    outs=[bass.AP(cc_out, 0, [[SL, NCORES], [1, SL]])],
                )

            if phases >= 2 and not int(os.environ.get("GCN_NO_SCATTER", "0")):
                # one DMA scatters all cores' h2 into the table rows:
                # position q = 1024t + 128c + j, row elements [0:16)
                dst = bass.AP(
                    h2_dram,
                    0,
                    [[P * ROWE, NCORES], [ROWE, P], [1024 * ROWE, NT], [1, FO]],
                )
                nc.sync.dma_start(out=dst, in_=cc_out[:, :, :])
                tc.strict_bb_all_engine_barrier()

            # ---------------- Phase D: layer-2 aggregation + log_softmax -----
            if phases >= 3:
                with tc.tile_pool(name="ep2", bufs=4) as ep2pool:

                    def epi2(t, pt):
                        tmp = ep2pool.tile([P, FO], dt.float32, tag="tmp2")
                        nc.scalar.activation(
                            out=tmp[:],
                            in_=pt[:],
                            func=Act.Copy,
                            scale=dinvs[:, t : t + 1],
                        )
                        nc.vector.tensor_tensor(
                            out=tmp[:], in0=tmp[:], in1=b2r[:], op=Alu.add
                        )
                        mx = ep2pool.tile([P, 1], dt.float32, tag="mx")
                        nc.vector.reduce_max(
                            out=mx[:], in_=tmp[:], axis=mybir.AxisListType.X,
                            negate=True,
                        )
                        ex = ep2pool.tile([P, FO], dt.float32, tag="ex")
                        nc.scalar.activation(
                            out=ex[:], in_=tmp[:], func=Act.Exp, bias=mx[:, 0:1]
                        )
                        sm = ep2pool.tile([P, 1], dt.float32, tag="sm")
                        nc.vector.reduce_sum(
                            out=sm[:], in_=ex[:], axis=mybir.AxisListType.X
                        )
                        lg = ep2pool.tile([P, 1], dt.float32, tag="lg")
                        nc.scalar.activation(out=lg[:], in_=sm[:], func=Act.Ln)
                        nc.vector.tensor_scalar(
                            out=outst[:, t * FO : (t + 1) * FO],
                            in0=tmp[:],
                            scalar1=mx[:, 0:1],
                            scalar2=lg[:, 0:1],
                            op0=Alu.add,
                            op1=Alu.subtract,
                        )

                    aggregation(h2_dram, ROWE, dt.bfloat16, ident, FO, FO, epi2)
                nc.sync.dma_start(out=out_e[:, :], in_=outst[:])
            else:
                nc.vector.memset(outst[:], 0.0)
                nc.sync.dma_start(out=out_e[:, :], in_=outst[:])

    _legalize_waits(nc)
    return nc


def kernel(x, edge_index, W1, b1, W2, b2, _trace=False, _trace_kwargs=None):
    in_maps, chunks, norder = _preprocess(x, edge_index, W1, b1, W2, b2)
    key = tuple(chunks)
    if key not in _CACHE:
        _CACHE[key] = _build(chunks)
    nc = _CACHE[key]

    res = run_bass_kernel_spmd(
        nc,
        in_maps,
        core_ids=list(range(NCORES)),
        trace=_trace,
        **(_trace_kwargs or {}),
    )
    out = np.empty((N, FO), dtype=np.float32)
    for c in range(NCORES):
        o = np.asarray(res.results[c]["out"], dtype=np.float32)
        o = o.reshape(P, NT, FO)  # [lane j, tile t, f]
        for t in range(NT):
            q0 = 1024 * t + P * c
            nodes = norder[q0 : q0 + P]
            m = nodes >= 0
            out[nodes[m]] = o[m, t]
    kernel._last_result = res
    return out
